# revision 14
# baseline (speedup 1.0000x reference)
"""HEPT block-local RBF attention on 8 TRN2 NeuronCores.

Reference computation, per independent 128x128 block:
  S[i,j] = q_i . k_j - 0.5||q_i||^2 - 0.5||k_j||^2   (= -0.5||q_i - k_j||^2 <= 0)
  A = exp(min(S, 0));  O = A @ V

Shapes: q,k [4,8,64,128,67] f32, v [4,8,64,128,64] f32.
B*H = 32 (b,h) slices are sharded 4-per-core across 8 cores (fully data
parallel, no collectives). 256 blocks per core.

Kernel design (per core), v2:
- The 67-dim contraction is split 64 (main) + 3 (tail). The 64-row mains of
  the two blocks of a pair are stacked vertically (partitions 0-63 / 64-127)
  so main DMAs engage all 16 SBUF AXI ports. The matmul for the upper block
  uses base partition 64 (tile_position auto-inferred).
- Norm terms are folded out of the device kernel entirely (v3):
  exp(qk+nk+nq) @ V == exp(nq_i) * (exp(qk) @ (exp(nk_j)*V)). The host
  pre-scales V' = exp(nk)*V (f64-exact) and post-scales output rows by
  exp(nq_i). The device computes A' = exp(q.k) only, so the tail matmul
  carries just the 3 real tail dims per block (K=6 per pair, block-diag)
  and the norm path has zero device cost and f64 accuracy.
- exp runs bias-free on ScalarE, one ACTIVATE per PSUM bank [128,512].
  min(.,0) clamp dropped: it cancels exactly in the refactored product.
- A' is written in bf16 (max |qk| ~ 48 -> e^48 well within bf16 range) and
  used as the stationary operand of the AV matmul against V' (bf16).
  VectorE copies PSUM f32 -> bf16 out, [128,512] at a time.
- DMA is chunked: 8 quads (32 blocks) per chunk, one dma_start per tensor
  per chunk (mains 1MB, tails 196KB, v 512KB, out 512KB) instead of
  per-quad/per-pair transfers. Tail tiles rotate their partition base
  0/32/64/96 across chunks to spread the 16-partition transfers over all
  SDMA engines.
"""

import numpy as np
import ml_dtypes

B, H, NB, BS = 4, 8, 64, 128
DQK, DV = 67, 64
N_CORES = 8
BH_PER_CORE = B * H // N_CORES          # 4
BLOCKS_PER_CORE = BH_PER_CORE * NB      # 256
PAIRS = BLOCKS_PER_CORE // 2            # 128
QUADS = BLOCKS_PER_CORE // 4            # 64
DMAIN = 64                              # d rows in the paired main tile
DTAIL = DQK - DMAIN                     # 3
CH = 8                                  # quads per DMA chunk
NCH = QUADS // CH                       # 8 chunks per core
KT = 6                                  # tail-matmul contraction rows per pair

_compiled = None


def _build_program(reps=1):
    from concourse import bacc, mybir
    from concourse.tile import TileContext

    fp16 = mybir.dt.float16
    bf16 = mybir.dt.bfloat16
    f32 = mybir.dt.float32
    Exp = mybir.ActivationFunctionType.Exp

    nc = bacc.Bacc(
        "TRN2",
        target_bir_lowering=False,
        debug=False,
        num_devices=N_CORES,
    )

    mains_t = nc.dram_tensor("mains", [NCH, 128, CH * 512], fp16, kind="ExternalInput")
    tails_t = nc.dram_tensor("tails", [NCH, KT, CH * 768], fp16, kind="ExternalInput")
    v_t = nc.dram_tensor("v", [NCH, 128, CH * 256], bf16, kind="ExternalInput")
    out_t = nc.dram_tensor("out", [NCH, 128, CH * 256], bf16, kind="ExternalOutput")

    with TileContext(nc) as tc:
        with (
            tc.tile_pool(name="mainp", bufs=3) as mainp,
            tc.tile_pool(name="tailp", bufs=3) as tailp,
            tc.tile_pool(name="vp", bufs=3) as vp,
            tc.tile_pool(name="outp", bufs=2) as outp,
            tc.tile_pool(name="a0p", bufs=4) as a0p,
            tc.tile_pool(name="psS", bufs=3, space="PSUM") as psSp,
            tc.tile_pool(name="psO", bufs=2, space="PSUM") as psOp,
        ):
          for _rep in range(reps):
            # Software-pipelined emission (engine streams execute in order):
            #   iter G: [chunk-boundary in-DMAs] S(G)+exp(G) on PE/ACT,
            #           then AV(G-1)+copy(G-1), then out-DMA(chunk-1).
            # PE never waits on ACT(G) (it runs S(G+1) first), and SP issues
            # chunk c+1's input DMAs before chunk c's output DMA.
            NG = NCH * (CH // 2)  # 2-quad groups per rep
            PIPE = 2  # AV/copy trail the S/exp front by this many groups
            tiles = {}
            pend = []  # [(a0, vt_of_group, g_in_chunk, ot), ...]
            for G in range(NG + PIPE):
                if G < NG:
                    c = G // (CH // 2)
                    gc = G % (CH // 2)
                    if gc == 0:
                        mt = mainp.tile([128, CH * 512], fp16)
                        nc.sync.dma_start(out=mt, in_=mains_t[c])
                        tt = tailp.tile([128, CH * 768], fp16)
                        tb = 32 * (c % 3)
                        tsl = tt[tb : tb + KT, :]
                        nc.sync.dma_start(out=tsl, in_=tails_t[c])
                        vt = vp.tile([128, CH * 256], bf16)
                        nc.sync.dma_start(out=vt, in_=v_t[c])
                        ot = outp.tile([128, CH * 256], bf16)
                        tiles[c] = (mt, tsl, vt, ot)
                    mt, tsl, vt, ot = tiles[c]
                    ps = psSp.tile([128, 1024], f32)
                    a0 = a0p.tile([128, 1024], bf16)
                    for qq2 in range(2):
                        qq = 2 * gc + qq2
                        for pp in range(2):
                            mcol = 512 * qq + 256 * pp
                            scol = 512 * qq2 + 256 * pp
                            # Per-BLOCK accumulation groups, tail FIRST:
                            # start=True clears has_written for the WHOLE
                            # bank, so groups sharing a bank must be strictly
                            # sequential (start ... stop before the next
                            # start), and HW rejects mixed-width groups
                            # (N=256 start + N=128 accumulate crashes). Each
                            # block: K=16 tail+norms matmul starts its 128
                            # cols, K=64 main accumulates and stops.
                            tcol = 384 * (2 * qq + pp)
                            for X in range(2):
                                nc.tensor.matmul(
                                    ps[:, scol + 128 * X : scol + 128 * X + 128],
                                    lhsT=tsl[:, tcol + 256 : tcol + 384],
                                    rhs=tsl[:, tcol + 128 * X : tcol + 128 * X + 128],
                                    start=True,
                                    stop=False,
                                )
                                nc.tensor.matmul(
                                    ps[:, scol + 128 * X : scol + 128 * X + 128],
                                    lhsT=mt[64 * X : 64 * X + 64, mcol + 128 : mcol + 256],
                                    rhs=mt[64 * X : 64 * X + 64, mcol : mcol + 128],
                                    start=False,
                                    stop=True,
                                )
                    # ACTIVATE must not cross a PSUM bank boundary: two
                    # N=512 exps per 2-quad tile.
                    nc.scalar.activation(a0[:, 0:512], ps[:, 0:512], Exp)
                    nc.scalar.activation(a0[:, 512:1024], ps[:, 512:1024], Exp)
                    pend.append((a0, vt, gc, ot))

                if len(pend) > (PIPE if G < NG else 0):
                    a0p_, vtp_, gcp_, otp_ = pend.pop(0)
                    po = psOp.tile([128, 512], f32)
                    for w in range(8):  # blocks in the 2-quad group
                        qq2, u = divmod(w, 4)
                        qq = 2 * gcp_ + qq2
                        nc.tensor.matmul(
                            po[:, 64 * w : 64 * w + 64],
                            lhsT=a0p_[:, 128 * w : 128 * w + 128],
                            rhs=vtp_[:, 256 * qq + 64 * u : 256 * qq + 64 * u + 64],
                            start=True,
                            stop=True,
                        )
                    nc.vector.tensor_copy(
                        out=otp_[:, 512 * gcp_ : 512 * gcp_ + 512], in_=po
                    )
                    if gcp_ % 2 == 1:
                        # Output DMA per 2 groups (256KB) on SWDGE (Pool):
                        # SP's in-order stream carries only input DMAs and
                        # never stalls on compute; finer granularity shortens
                        # the end-of-kernel pipeline drain.
                        cp = ((G - PIPE) // (CH // 2))
                        h = gcp_ // 2
                        nc.gpsimd.dma_start(
                            out=out_t[cp][:, 1024 * h : 1024 * h + 1024],
                            in_=otp_[:, 1024 * h : 1024 * h + 1024],
                        )
    nc.compile()
    return nc


def _get_program():
    global _compiled
    if _compiled is None:
        _compiled = _build_program()
    return _compiled


def _prep_core_inputs(qc, kc, vc):
    """qc,kc: [256,128,67] f32; vc: [256,128,64] f32 -> (in_map dict, exp_nq).

    Norms are folded OUT of the device kernel:
      exp(qk + nk_j + nq_i) @ V  ==  exp(nq_i) * (exp(qk + 0) @ (exp(nk_j)*V))
    so the device computes A' = exp(q.k) (pure dot product, no norm rows) and
    multiplies by V' = exp(nk)*V (host, f64-exact); the host post-scales the
    output rows by exp(nq_i). Tail contraction drops to the 3 real dims.
    """
    qT = np.ascontiguousarray(qc.transpose(0, 2, 1))  # [256, 67, 128]
    kT = np.ascontiguousarray(kc.transpose(0, 2, 1))

    # mains: pair regions [128, 256] = [2 x 64 d-rows, qT | kT], chunked
    qm = qT[:, :DMAIN, :].reshape(PAIRS, 2 * DMAIN, BS)
    km = kT[:, :DMAIN, :].reshape(PAIRS, 2 * DMAIN, BS)
    mains = np.concatenate([qm, km], axis=2).astype(np.float16)  # [PAIRS,128,256]
    mains = np.ascontiguousarray(
        mains.reshape(NCH, 2 * CH, 128, 256).transpose(0, 2, 1, 3)
        .reshape(NCH, 128, CH * 512)
    )

    nk = -0.5 * np.sum(kc.astype(np.float64) ** 2, axis=2)  # [256, 128] over j
    nq = -0.5 * np.sum(qc.astype(np.float64) ** 2, axis=2)  # [256, 128] over i

    # tails: per pair [KT=6, 384] = rhs [6,256] | lhsT [6,128], block-diagonal
    qt = qT[:, DMAIN:, :].astype(np.float16)  # [256, 3, 128]
    kt = kT[:, DMAIN:, :].astype(np.float16)
    T = np.zeros((PAIRS, KT, 384), np.float16)
    for X in range(2):  # block in pair
        r = 3 * X
        T[:, r : r + 3, 128 * X : 128 * X + 128] = qt[X::2]
        T[:, r : r + 3, 256:384] = kt[X::2]
    tails = np.ascontiguousarray(
        T.reshape(NCH, 2 * CH, KT, 384).transpose(0, 2, 1, 3)
        .reshape(NCH, KT, CH * 768)
    )

    # v' = exp(nk_j) * v, f64-exact, then bf16: [QUADS, 128, 256] chunked
    vs = vc.astype(np.float64) * np.exp(nk)[:, :, None]  # [256, 128, 64]
    vq = vs.reshape(QUADS, 4, BS, DV).transpose(0, 2, 1, 3).reshape(QUADS, BS, 4 * DV)
    vq = vq.astype(ml_dtypes.bfloat16)
    v = np.ascontiguousarray(
        vq.reshape(NCH, CH, 128, 256).transpose(0, 2, 1, 3).reshape(NCH, 128, CH * 256)
    )

    return {"mains": mains, "tails": tails, "v": v}, np.exp(nq)  # [256,128] f64


def prep_in_maps(query, key, value):
    q = np.asarray(query, np.float32).reshape(B * H, NB, BS, DQK)
    k = np.asarray(key, np.float32).reshape(B * H, NB, BS, DQK)
    v = np.asarray(value, np.float32).reshape(B * H, NB, BS, DV)
    in_maps, scales = [], []
    for c in range(N_CORES):
        sl = slice(c * BH_PER_CORE, (c + 1) * BH_PER_CORE)
        qc = q[sl].reshape(BLOCKS_PER_CORE, BS, DQK)
        kc = k[sl].reshape(BLOCKS_PER_CORE, BS, DQK)
        vc = v[sl].reshape(BLOCKS_PER_CORE, BS, DV)
        im, sc = _prep_core_inputs(qc, kc, vc)
        in_maps.append(im)
        scales.append(sc)
    return in_maps, scales


def assemble_output(results, scales):
    """results: per-core dicts with 'out' [NCH,128,CH*256] bf16; scales: exp(nq)."""
    out = np.empty((B * H, NB, BS, DV), np.float32)
    for c in range(N_CORES):
        oc = np.asarray(results[c]["out"]).astype(np.float64)
        oc = oc.reshape(NCH, BS, CH, 4, DV).transpose(0, 2, 3, 1, 4)
        oc = oc.reshape(BLOCKS_PER_CORE, BS, DV) * scales[c][:, :, None]
        out[c * BH_PER_CORE : (c + 1) * BH_PER_CORE] = (
            oc.astype(np.float32).reshape(BH_PER_CORE, NB, BS, DV)
        )
    return out.reshape(B, H, NB, BS, DV)


def run(query, key, value, trace=False, **kwargs):
    from concourse import bass_utils

    nc = _get_program()
    in_maps, scales = prep_in_maps(query, key, value)
    res = bass_utils.run_bass_kernel_spmd(
        nc, in_maps, core_ids=list(range(N_CORES)), trace=trace, **kwargs
    )
    out = assemble_output(res.results, scales)
    return out, res


def kernel(query, key, value):
    out, _ = run(query, key, value)
    return out

